# revision 15
# baseline (speedup 1.0000x reference)
"""DeeperGCN forward on 8 TRN2 NeuronCores (Bass/Tile).

Sharding: dst-partitioned graph parallel. Core k owns a contiguous range of
12500 original nodes (relabeled + padded to 12800 = 100 blocks of 128).
Edges live on the core owning their dst. Per layer:
  x = relu(LN(h)) locally -> AllGather x to every core's DRAM -> per-edge
  gather x[src] via indirect DMA -> messages + softmax-free segment
  aggregation (one-hot matmuls into PSUM) -> GENConv MLP -> h += y.
Softmax uses no segment-max: logits are bounded (LN output <= sqrt(63)), so
exp() cannot overflow, and e/s is shift-invariant so results match the
reference to fp rounding.
"""
import json
import math
import numpy as np
import ml_dtypes

import concourse.bass as bass
import concourse.mybir as mybir
import concourse.tile as tile
from concourse.bass_utils import run_bass_kernel_spmd
from concourse.masks import make_identity

# ---- problem constants (hardcoded per contract) ----
N_NODES = 100000
N_EDGES = 1200000
NODE_DIM = 128
EDGE_DIM = 8
HID = 64
OUT_DIM = 112
L_LAYERS = 7
EPS_MSG = 1e-7
LN_EPS = 1e-5

N_CORES = 8
N_PER_CORE = 12500          # original nodes per core
NBLK = 100                  # dst blocks per core (128 dsts each)
NLOC = NBLK * 128           # 12800 padded local nodes
NFULL = N_CORES * NLOC      # 102400 padded global rows in gathered x
NCHB = 12                   # edge chunks per block
NCH = NBLK * NCHB           # 1200 chunks per core
ECAP = NCH * 128            # 153600 edge slots per core
F32 = mybir.dt.float32
BF16 = mybir.dt.bfloat16
I32 = mybir.dt.int32


def _split_multi_waits(bir_bytes: bytes) -> bytes:
    """Walrus in this container allows only ONE semaphore wait per
    instruction: hoist extra waits onto same-engine NoOps."""
    d = json.loads(bir_bytes)
    ctr = 0
    for f in d["functions"]:
        for blk in f["blocks"]:
            insts = blk["instructions"]
            out = []
            changed = False
            for inst in insts:
                si = inst.get("sync_info")
                if si:
                    waits = si.get("on_wait") or []
                    if len(waits) > 1:
                        changed = True
                        for w in waits[:-1]:
                            ctr += 1
                            out.append({
                                "debug": inst.get("debug", 0),
                                "engine": inst["engine"],
                                "ins": [], "outs": [],
                                "name": f"I-wsplit-{ctr}",
                                "opcode": "NoOp",
                                "sync_info": {"on_wait": [w], "on_update": []},
                            })
                        si["on_wait"] = waits[-1:]
                out.append(inst)
            if changed:
                blk["instructions"] = out
    return json.dumps(d).encode()


def _install_wait_split(nc):
    orig = nc.to_json_bytes
    nc.to_json_bytes = lambda: _split_multi_waits(orig())


# ---------------------------------------------------------------- host prep
def _pack_blocks(degrees):
    """Greedy bin packing of dsts into NBLK blocks: <=128 dsts and <=NCHB*128
    edges per block. Returns per-block list of dst-local ids."""
    order = np.argsort(-degrees, kind="stable")
    cap_e = NCHB * 128
    blk_edges = np.zeros(NBLK, np.int64)
    blk_cnt = np.zeros(NBLK, np.int64)
    blocks = [[] for _ in range(NBLK)]
    for d in order:
        deg = degrees[d]
        # block with most remaining edge room among those with a free dst slot
        room = cap_e - blk_edges
        room[blk_cnt >= 128] = -1
        b = int(np.argmax(room))
        assert room[b] >= deg, "block capacity exceeded"
        blocks[b].append(int(d))
        blk_edges[b] += deg
        blk_cnt[b] += 1
    return blocks


def _prep_core(k, src, dst, edge_feats):
    """Build per-core arrays for core k."""
    sel = np.where((dst >= k * N_PER_CORE) & (dst < (k + 1) * N_PER_CORE))[0]
    dl = dst[sel] - k * N_PER_CORE          # dst local 0..12499
    s_orig = src[sel]
    ef = edge_feats[sel]

    deg = np.bincount(dl, minlength=N_PER_CORE)
    blocks = _pack_blocks(deg)

    # relabeling: new local id of original-local dst  = block*128 + row
    relab = np.full(NLOC, -1, np.int64)      # new -> orig (for output unperm)
    orig2new = np.full(N_PER_CORE, -1, np.int64)
    for b, ids in enumerate(blocks):
        for r, d in enumerate(ids):
            nn = b * 128 + r
            orig2new[d] = nn
            relab[nn] = d
    assert (orig2new >= 0).all()

    # group edges by block, lay into NCHB*128 slots each
    newdl = orig2new[dl]                      # new local dst id per edge
    border = np.argsort(newdl, kind="stable")
    s_orig = s_orig[border]
    ef = ef[border]
    newdl = newdl[border]
    eblk = newdl // 128
    erow = newdl % 128

    assert len(sel) <= ECAP, f"core {k}: {len(sel)} edges > capacity {ECAP}"
    src_arr = np.zeros((128, NCH), np.int32)
    smat = np.zeros((NBLK, 128, NCHB * 128), ml_dtypes.bfloat16)
    # row 8 is a constant 1 for the folded b_edge bias (only on real edges'
    # slots; pad slots keep 0 so padded eh rows stay 0)
    ef_arr = np.zeros((NCH, EDGE_DIM + 1, 128), np.float32)

    # map src original -> padded global row (owner core * NLOC + new local id)
    oc = s_orig // N_PER_CORE
    # orig-local -> new-local mapping of the OWNER core is needed; computed later
    # globally (two-pass). Here keep owner core + orig local.
    s_loc = s_orig % N_PER_CORE

    start = np.zeros(NBLK + 1, np.int64)
    np.add.at(start[1:], eblk, 1)
    start = np.cumsum(start)
    meta_src = []  # (slot_index, owner_core, owner_local) for later fixup
    for b in range(NBLK):
        lo, hi = start[b], start[b + 1]
        n = hi - lo
        assert n <= NCHB * 128
        idx = np.arange(n)
        cc = idx // 128           # chunk in block
        pp = idx % 128            # partition
        slot = (b * NCHB + cc)
        src_arr[pp, slot] = 0     # placeholder, fixed after global relabel
        meta_src.append((pp, slot, oc[lo:hi], s_loc[lo:hi]))
        smat[b, pp, cc * 128 + erow[lo:hi]] = 1.0
        ef_arr[b * NCHB + cc, :EDGE_DIM, pp] = ef[lo:hi]
        ef_arr[b * NCHB + cc, EDGE_DIM, pp] = 1.0
    return dict(relab=relab, orig2new=orig2new, src_arr=src_arr, smat=smat,
                ef_arr=ef_arr, meta_src=meta_src)


def host_prep(node_feats, edge_feats, src, dst):
    cores = [_prep_core(k, src, dst, edge_feats) for k in range(N_CORES)]
    # global src fixup now that every core's orig2new is known
    for c in cores:
        for pp, slot, ocs, olocs in c["meta_src"]:
            newloc = np.empty(len(ocs), np.int64)
            for k in range(N_CORES):
                m = ocs == k
                if m.any():
                    newloc[m] = cores[k]["orig2new"][olocs[m]]
            c["src_arr"][pp, slot] = (ocs * NLOC + newloc).astype(np.int32)
        del c["meta_src"]
    # node features transposed + relabeled + padded per core
    for k, c in enumerate(cores):
        nf = np.zeros((NLOC, NODE_DIM), np.float32)
        valid = c["relab"] >= 0
        nf[valid] = node_feats[k * N_PER_CORE + c["relab"][valid]]
        c["nfT"] = np.ascontiguousarray(nf.T)            # [128, NLOC]
    return cores


# ---------------------------------------------------------------- device build
def build_bass(weights, n_layers=L_LAYERS):
    (W_node, b_node, W_edge, b_edge, betas, W1, b1, ln1_g, ln1_b,
     W2, b2, norm_g, norm_b, W_out, b_out) = weights
    nc = bass.Bass("TRN2", target_bir_lowering=False, debug=False,
                   num_devices=N_CORES)

    nfT_d = nc.dram_tensor("nft", [NODE_DIM, NLOC], F32, kind="ExternalInput")
    ef_d = nc.dram_tensor("ef", [NCH, EDGE_DIM + 1, 128], F32, kind="ExternalInput")
    sidx_d = nc.dram_tensor("sidx", [128, NCH], I32, kind="ExternalInput")
    smat_d = nc.dram_tensor("smat", [NBLK, 128, NCHB * 128], BF16,
                            kind="ExternalInput")
    wb_d = {}
    wedge_stack = np.concatenate([W_edge, b_edge[None, :]], axis=0)  # [9, HID]
    for name, arr in [
        ("wnode", W_node), ("wedge", wedge_stack), ("w1", W1), ("w2", W2),
        ("wout", W_out),
        ("bnode", np.broadcast_to(b_node, (128, HID))),
        ("b1", np.broadcast_to(b1[:, None, :], (n_layers if False else L_LAYERS, 128, 2 * HID))),
        ("b2", np.broadcast_to(b2[:, None, :], (L_LAYERS, 128, HID))),
        ("l1g", np.broadcast_to(ln1_g[:, None, :], (L_LAYERS, 128, 2 * HID))),
        ("l1b", np.broadcast_to(ln1_b[:, None, :], (L_LAYERS, 128, 2 * HID))),
        ("ng", np.broadcast_to(norm_g[:, None, :], (L_LAYERS, 128, HID))),
        ("nb", np.broadcast_to(norm_b[:, None, :], (L_LAYERS, 128, HID))),
        ("bout", np.broadcast_to(b_out, (128, OUT_DIM))),
    ]:
        a = np.ascontiguousarray(arr, np.float32)
        wb_d[name] = nc.dram_tensor(name, list(a.shape), F32, kind="ExternalInput")
        wb_d[f"_{name}_np"] = a
    out_d = nc.dram_tensor("out", [NLOC, OUT_DIM], F32, kind="ExternalOutput")

    with tile.TileContext(nc) as tc:
        with tc.tile_pool(name="persist", bufs=1) as pp, \
             tc.tile_pool(name="dram", bufs=1, space="DRAM") as dram, \
             tc.tile_pool(name="work", bufs=2) as wk, \
             tc.tile_pool(name="nodeops", bufs=3) as nop, \
             tc.tile_pool(name="pseg", bufs=2, space="PSUM") as pseg, \
             tc.tile_pool(name="pmlp", bufs=1, space="PSUM") as pmlp:

            # ---- persistent SBUF state ----
            h_sb = pp.tile([128, NBLK, HID], F32)           # node-major h
            x_sb = pp.tile([128, NBLK, HID], F32)           # x = relu(LN(h))
            sidx_sb = pp.tile([128, NCH], I32)
            nc.sync.dma_start(out=sidx_sb[:], in_=sidx_d[:, :])
            ident = pp.tile([128, 128], F32)
            make_identity(nc, ident[:])
            eps_sb = pp.tile([128, 1], F32)
            nc.vector.memset(eps_sb[:], LN_EPS)

            wnode_sb = pp.tile([128, HID], F32)
            nc.sync.dma_start(out=wnode_sb[:], in_=wb_d["wnode"][:, :])
            wedge_sb = pp.tile([EDGE_DIM + 1, HID], F32)
            nc.sync.dma_start(out=wedge_sb[:], in_=wb_d["wedge"][:, :])
            w1_sb = pp.tile([HID, L_LAYERS, 2 * HID], F32)
            nc.sync.dma_start(out=w1_sb[:], in_=wb_d["w1"][:, :, :].rearrange("l k m -> k l m"))
            w2_sb = pp.tile([2 * HID, L_LAYERS, HID], F32)
            nc.sync.dma_start(out=w2_sb[:], in_=wb_d["w2"][:, :, :].rearrange("l k m -> k l m"))
            wout_sb = pp.tile([HID, OUT_DIM], F32)
            nc.sync.dma_start(out=wout_sb[:], in_=wb_d["wout"][:, :])
            bias_sb = {}
            for nm, sh in [("bnode", [128, HID]),
                           ("bout", [128, OUT_DIM])]:
                bias_sb[nm] = pp.tile(sh, F32, name=f"bs_{nm}")
                nc.sync.dma_start(out=bias_sb[nm][:], in_=wb_d[nm][:, :])
            for nm, dd in [("b1", 2 * HID), ("l1g", 2 * HID), ("l1b", 2 * HID),
                           ("b2", HID), ("ng", HID), ("nb", HID)]:
                bias_sb[nm] = pp.tile([128, L_LAYERS, dd], F32, name=f"bs_{nm}")
                nc.sync.dma_start(out=bias_sb[nm][:],
                                  in_=wb_d[nm][:, :, :].rearrange("l p d -> p l d"))

            # ---- DRAM internals ----
            eh_dram = dram.tile([NBLK, 128, NCHB * HID], F32)
            # collective buffers: Shared outputs may only be written once, so
            # one pair per layer
            xins = [dram.tile([NLOC, HID], F32, name=f"xin{i}")
                    for i in range(n_layers)]
            xfulls = [dram.tile([NFULL, HID], F32, addr_space="Shared",
                                name=f"xfull{i}") for i in range(n_layers)]

            # ---- phase 0a: h0 = nf @ W_node + b_node ----
            with tc.tile_pool(name="p0", bufs=1, space="PSUM") as p0, \
                 tc.tile_pool(name="w0", bufs=3) as w0:
                for b in range(NBLK):
                    nft_t = w0.tile([128, 128], F32, name="nft_t")
                    nc.sync.dma_start(out=nft_t[:], in_=nfT_d[:, b * 128:(b + 1) * 128])
                    ps = p0.tile([128, HID], F32, name="ps0")
                    nc.tensor.matmul(out=ps[:], lhsT=nft_t[:], rhs=wnode_sb[:],
                                     start=True, stop=True)
                    nc.vector.tensor_tensor(out=h_sb[:, b, :], in0=ps[:],
                                            in1=bias_sb["bnode"][:],
                                            op=mybir.AluOpType.add)
                # ---- phase 0b: eh = ef @ [W_edge; b_edge] -> eh_dram ----
                for b in range(NBLK):
                    eft = w0.tile([EDGE_DIM + 1, NCHB, 128], F32, name="eft")
                    nc.sync.dma_start(
                        out=eft[:, :, :],
                        in_=ef_d[b * NCHB:(b + 1) * NCHB, :, :].rearrange("c k p -> k c p"))
                    ehs = w0.tile([128, NCHB * HID], F32, name="ehs")
                    for half in range(2):
                        pse = p0.tile([128, (NCHB // 2) * HID], F32, name="pse")
                        for ci in range(NCHB // 2):
                            cabs = half * (NCHB // 2) + ci
                            nc.tensor.matmul(
                                out=pse[:, ci * HID:(ci + 1) * HID],
                                lhsT=eft[:, cabs, :],
                                rhs=wedge_sb[:], start=True, stop=True)
                        nc.vector.tensor_copy(
                            out=ehs[:, half * (NCHB // 2) * HID:(half + 1) * (NCHB // 2) * HID],
                            in_=pse[:])
                    nc.sync.dma_start(out=eh_dram[b, :, :], in_=ehs[:])

            # ================= layers =================
            for li in range(n_layers):
                beta = float(betas[li])
                xin = xins[li]
                xfull = xfulls[li]
                # ---- x = relu(LN(h)) per block; write pure x to xin ----
                for b in range(NBLK):
                    hb = h_sb[:, b, :]
                    st = nop.tile([128, 6], F32, name="st")
                    nc.vector.bn_stats(out=st[:], in_=hb)
                    mv = nop.tile([128, 2], F32, name="mv")
                    nc.vector.bn_aggr(out=mv[:], in_=st[:])
                    lnv = nop.tile([128, 1], F32, name="lnv")
                    nc.scalar.activation(out=lnv[:], in_=mv[:, 1:2],
                                         func=mybir.ActivationFunctionType.Ln,
                                         bias=eps_sb[:])
                    rstd = nop.tile([128, 1], F32, name="rstd")
                    nc.scalar.activation(out=rstd[:], in_=lnv[:],
                                         func=mybir.ActivationFunctionType.Exp,
                                         scale=-0.5)
                    xn = nop.tile([128, HID], F32, name="xn")
                    nc.vector.tensor_scalar(out=xn[:], in0=hb,
                                            scalar1=mv[:, 0:1], scalar2=rstd[:],
                                            op0=mybir.AluOpType.subtract,
                                            op1=mybir.AluOpType.mult)
                    nc.vector.tensor_tensor(out=xn[:], in0=xn[:],
                                            in1=bias_sb["ng"][:, li, :],
                                            op=mybir.AluOpType.mult)
                    nc.vector.tensor_tensor(out=xn[:], in0=xn[:],
                                            in1=bias_sb["nb"][:, li, :],
                                            op=mybir.AluOpType.add)
                    nc.scalar.activation(out=x_sb[:, b, :], in_=xn[:],
                                         func=mybir.ActivationFunctionType.Relu)
                    nc.sync.dma_start(out=xin[b * 128:(b + 1) * 128, :],
                                      in_=x_sb[:, b, :])
                # ---- all-gather x ----
                nc.gpsimd.collective_compute(
                    "AllGather", mybir.AluOpType.bypass,
                    replica_groups=[list(range(N_CORES))],
                    ins=[xin[:, :].opt()], outs=[xfull[:, :].opt()])

                # ---- edge phase: per block, 12 chunks ----
                for b in range(NBLK):
                    eht = wk.tile([128, NCHB * HID], F32, name="eht")
                    nc.sync.dma_start(out=eht[:], in_=eh_dram[b, :, :])
                    smt = wk.tile([128, NCHB, 128], BF16, name="smt")
                    nc.sync.dma_start(
                        out=smt[:].rearrange("p c d -> p (c d)"),
                        in_=smat_d[b, :, :])
                    xg = wk.tile([128, NCHB, HID], F32, name="xg")
                    for c in range(NCHB):
                        nc.gpsimd.indirect_dma_start(
                            out=xg[:, c, :], out_offset=None,
                            in_=xfull[:, :],
                            in_offset=bass.IndirectOffsetOnAxis(
                                ap=sidx_sb[:, b * NCHB + c:b * NCHB + c + 1], axis=0))
                    z = wk.tile([128, NCHB * HID], F32, name="z")
                    nc.vector.tensor_tensor(out=z[:], in0=xg[:].rearrange("p c d -> p (c d)"),
                                            in1=eht[:], op=mybir.AluOpType.add)
                    mr = wk.tile([128, NCHB, HID], F32, name="mr")
                    nc.scalar.activation(out=mr[:].rearrange("p c d -> p (c d)"), in_=z[:],
                                         func=mybir.ActivationFunctionType.Relu)
                    em = wk.tile([128, NCHB, 2 * HID], BF16, name="em")
                    nc.scalar.activation(out=em[:, :, 0:HID], in_=mr[:],
                                         func=mybir.ActivationFunctionType.Exp,
                                         scale=beta)
                    nc.vector.tensor_tensor(out=em[:, :, HID:2 * HID], in0=mr[:],
                                            in1=em[:, :, 0:HID],
                                            op=mybir.AluOpType.mult)
                    ps = pseg.tile([128, 2 * HID], F32, name="ps")
                    for c in range(NCHB):
                        nc.tensor.matmul(out=ps[:], lhsT=smt[:, c, :],
                                         rhs=em[:, c, :],
                                         start=(c == 0), stop=(c == NCHB - 1))
                    # ---- aggregation -> y ----
                    sden = nop.tile([128, HID], F32, name="sden")
                    nc.vector.tensor_scalar_max(out=sden[:], in0=ps[:, 0:HID],
                                                scalar1=0.5)
                    rs = nop.tile([128, HID], F32, name="rs")
                    nc.vector.reciprocal(out=rs[:], in_=sden[:])
                    y = nop.tile([128, HID], F32, name="y")
                    nc.vector.tensor_tensor(out=y[:], in0=ps[:, HID:2 * HID],
                                            in1=rs[:], op=mybir.AluOpType.mult)
                    nc.vector.tensor_scalar_add(out=y[:], in0=y[:], scalar1=EPS_MSG)
                    nc.vector.tensor_tensor(out=y[:], in0=y[:], in1=x_sb[:, b, :],
                                            op=mybir.AluOpType.add)
                    # ---- MLP: z1 = relu(LN1(y@W1+b1)); h += z1@W2 + b2 ----
                    pyt = pmlp.tile([HID, 128], F32, name="pyt", tag="ptr", bufs=1)
                    nc.tensor.transpose(out=pyt[:], in_=y[:], identity=ident[:])
                    yt = nop.tile([HID, 128], F32, name="yt")
                    nc.vector.tensor_copy(out=yt[:], in_=pyt[:])
                    pm1 = pmlp.tile([128, 2 * HID], F32, name="pm1", tag="pmm", bufs=2)
                    nc.tensor.matmul(out=pm1[:], lhsT=yt[:], rhs=w1_sb[:, li, :],
                                     start=True, stop=True)
                    z0 = nop.tile([128, 2 * HID], F32, name="z0")
                    nc.vector.tensor_tensor(out=z0[:], in0=pm1[:],
                                            in1=bias_sb["b1"][:, li, :],
                                            op=mybir.AluOpType.add)
                    st1 = nop.tile([128, 6], F32, name="st1")
                    nc.vector.bn_stats(out=st1[:], in_=z0[:])
                    mv1 = nop.tile([128, 2], F32, name="mv1")
                    nc.vector.bn_aggr(out=mv1[:], in_=st1[:])
                    lnv1 = nop.tile([128, 1], F32, name="lnv1")
                    nc.scalar.activation(out=lnv1[:], in_=mv1[:, 1:2],
                                         func=mybir.ActivationFunctionType.Ln,
                                         bias=eps_sb[:])
                    rstd1 = nop.tile([128, 1], F32, name="rstd1")
                    nc.scalar.activation(out=rstd1[:], in_=lnv1[:],
                                         func=mybir.ActivationFunctionType.Exp,
                                         scale=-0.5)
                    z1 = nop.tile([128, 2 * HID], F32, name="z1")
                    nc.vector.tensor_scalar(out=z1[:], in0=z0[:],
                                            scalar1=mv1[:, 0:1], scalar2=rstd1[:],
                                            op0=mybir.AluOpType.subtract,
                                            op1=mybir.AluOpType.mult)
                    nc.vector.tensor_tensor(out=z1[:], in0=z1[:],
                                            in1=bias_sb["l1g"][:, li, :],
                                            op=mybir.AluOpType.mult)
                    nc.vector.tensor_tensor(out=z1[:], in0=z1[:],
                                            in1=bias_sb["l1b"][:, li, :],
                                            op=mybir.AluOpType.add)
                    nc.scalar.activation(out=z1[:], in_=z1[:],
                                         func=mybir.ActivationFunctionType.Relu)
                    pzt = pmlp.tile([128, 128], F32, name="pzt", tag="ptr", bufs=1)
                    nc.tensor.transpose(out=pzt[:], in_=z1[:], identity=ident[:])
                    zt = nop.tile([128, 128], F32, name="zt")
                    nc.vector.tensor_copy(out=zt[:], in_=pzt[:])
                    pm2 = pmlp.tile([128, HID], F32, name="pm2", tag="pmm", bufs=2)
                    nc.tensor.matmul(out=pm2[:], lhsT=zt[:], rhs=w2_sb[:, li, :],
                                     start=True, stop=True)
                    y2 = nop.tile([128, HID], F32, name="y2")
                    nc.vector.tensor_tensor(out=y2[:], in0=pm2[:],
                                            in1=bias_sb["b2"][:, li, :],
                                            op=mybir.AluOpType.add)
                    nc.vector.tensor_tensor(out=h_sb[:, b, :], in0=h_sb[:, b, :],
                                            in1=y2[:], op=mybir.AluOpType.add)

            # ================= final head =================
            for b in range(NBLK):
                hb = h_sb[:, b, :]
                st = nop.tile([128, 6], F32, name="stf")
                nc.vector.bn_stats(out=st[:], in_=hb)
                mv = nop.tile([128, 2], F32, name="mvf")
                nc.vector.bn_aggr(out=mv[:], in_=st[:])
                lnv = nop.tile([128, 1], F32, name="lnvf")
                nc.scalar.activation(out=lnv[:], in_=mv[:, 1:2],
                                     func=mybir.ActivationFunctionType.Ln,
                                     bias=eps_sb[:])
                rstd = nop.tile([128, 1], F32, name="rstdf")
                nc.scalar.activation(out=rstd[:], in_=lnv[:],
                                     func=mybir.ActivationFunctionType.Exp,
                                     scale=-0.5)
                xf = nop.tile([128, HID], F32, name="xf")
                nc.vector.tensor_scalar(out=xf[:], in0=hb,
                                        scalar1=mv[:, 0:1], scalar2=rstd[:],
                                        op0=mybir.AluOpType.subtract,
                                        op1=mybir.AluOpType.mult)
                nc.vector.tensor_tensor(out=xf[:], in0=xf[:],
                                        in1=bias_sb["ng"][:, 0, :],
                                        op=mybir.AluOpType.mult)
                nc.vector.tensor_tensor(out=xf[:], in0=xf[:],
                                        in1=bias_sb["nb"][:, 0, :],
                                        op=mybir.AluOpType.add)
                nc.scalar.activation(out=xf[:], in_=xf[:],
                                     func=mybir.ActivationFunctionType.Relu)
                pxt = pmlp.tile([HID, 128], F32, name="pxt", tag="ptr", bufs=1)
                nc.tensor.transpose(out=pxt[:], in_=xf[:], identity=ident[:])
                xt = nop.tile([HID, 128], F32, name="xt")
                nc.vector.tensor_copy(out=xt[:], in_=pxt[:])
                po = pmlp.tile([128, OUT_DIM], F32, name="po", tag="pmm", bufs=2)
                nc.tensor.matmul(out=po[:], lhsT=xt[:], rhs=wout_sb[:],
                                 start=True, stop=True)
                ob = nop.tile([128, OUT_DIM], F32, name="ob")
                nc.vector.tensor_tensor(out=ob[:], in0=po[:],
                                        in1=bias_sb["bout"][:],
                                        op=mybir.AluOpType.add)
                nc.sync.dma_start(out=out_d[b * 128:(b + 1) * 128, :], in_=ob[:])

    _install_wait_split(nc)
    wb_np = {k[1:-3]: v for k, v in wb_d.items() if k.startswith("_")}
    return nc, wb_np


def kernel(node_feats, edge_feats, src, dst, W_node, b_node, W_edge, b_edge,
           betas, W1, b1, ln1_g, ln1_b, W2, b2, norm_g, norm_b, W_out, b_out,
           n_layers=L_LAYERS, _cache={}):
    node_feats = np.asarray(node_feats, np.float32)
    edge_feats = np.asarray(edge_feats, np.float32)
    src = np.asarray(src, np.int32)
    dst = np.asarray(dst, np.int32)
    weights = (np.asarray(W_node, np.float32), np.asarray(b_node, np.float32),
               np.asarray(W_edge, np.float32), np.asarray(b_edge, np.float32),
               np.asarray(betas, np.float32), np.asarray(W1, np.float32),
               np.asarray(b1, np.float32), np.asarray(ln1_g, np.float32),
               np.asarray(ln1_b, np.float32), np.asarray(W2, np.float32),
               np.asarray(b2, np.float32), np.asarray(norm_g, np.float32),
               np.asarray(norm_b, np.float32), np.asarray(W_out, np.float32),
               np.asarray(b_out, np.float32))

    cores = host_prep(node_feats, edge_feats, src, dst)
    nc, wb_np = build_bass(weights, n_layers=n_layers)

    in_maps = []
    for k in range(N_CORES):
        c = cores[k]
        m = dict(nft=c["nfT"], ef=c["ef_arr"], sidx=c["src_arr"],
                 smat=np.asarray(c["smat"]))
        m.update({k2: v for k2, v in wb_np.items()})
        in_maps.append(m)

    res = run_bass_kernel_spmd(nc, in_maps, core_ids=list(range(N_CORES)))

    out = np.zeros((N_NODES, OUT_DIM), np.float32)
    for k in range(N_CORES):
        o = res.results[k]["out"]
        relab = cores[k]["relab"]
        valid = relab >= 0
        out[k * N_PER_CORE + relab[valid]] = o[valid]
    return out
